# revision 52
# baseline (speedup 1.0000x reference)
"""Causal multi-head self-attention with RoPE on 8 Trainium2 NeuronCores.

Sharding: batch (4) x head-half (2) -> 8 self-contained cores. Each core
computes Q/K/V projections for its 8 heads, RoPE, causal flash-style
attention (scores kept transposed [key, query] so probs feed the V matmul
with no on-device transpose), and a partial output projection over its 512
context features. The two partial outputs per batch are summed on host
(the "all-reduce after output projection" of the tensor-parallel split).

Device layout notes (v2):
- All matmul inputs are bf16 (PSUM accumulation stays fp32): same PE
  cycles/row as fp32r but half the DMA/SBUF footprint, and no >=256
  moving-dim requirement so diagonal score blocks are trimmed exactly.
- Softmax: scores masked additively via a PE-written bf16 -1e30 triangle,
  exponentiated without max-subtraction (scores bounded), per-query sums
  from a ones-column appended to V. The sum-reciprocal broadcast runs on
  the (otherwise idle) Pool engine via partition_broadcast, keeping both
  PE and ACT out of the normalization chain.
- The output projection accumulates in PSUM and is DMA'd straight to HBM
  (no SBUF staging copy).
- Cross-superblock software pipelining: the next superblock's Q/K/V
  projection groups and the previous superblock's output-projection
  groups are held in a work queue and interleaved one group per kj
  iteration into the attention loop, filling PE idle slots while the
  scalar engine's exp stream paces the attention inner loop.
- RoPE is evaluated as q*cos + swap(q)*(+-sin) where swap is a DVE
  stream_shuffle partition pair-swap and the sign is folded into the sin
  table.
"""

import sys

sys.path.insert(0, "/opt/trn_rl_repo")

import numpy as np

B, S_FULL, D, H = 4, 2048, 1024, 16
DK = 64  # head dim
HL = 8  # heads per core
DL = HL * DK  # 512 local features
ROPE_THETA = 10000.0
NEG = -1.0e30

_CACHE = {}


def _emit(nc, tc, tensors, S, reps=1):
    import concourse.tile as tile  # noqa: F401
    from concourse import mybir
    from contextlib import ExitStack

    f32 = mybir.dt.float32
    bf16 = mybir.dt.bfloat16
    AF = mybir.ActivationFunctionType
    SWAP = [i ^ 1 for i in range(32)]
    NSB = S // 512  # query super-blocks
    NKB = S // 128  # key blocks
    DEPTH = 3  # kj-iterations the V-matmul trails the score matmul by
    NSTEP = reps * NSB

    xT, wqT, wkT, wvT, woT = (
        tensors["xT"], tensors["wqT"], tensors["wkT"], tensors["wvT"], tensors["woT"],
    )
    tabs, maskT, ident = tensors["tabs"], tensors["maskT"], tensors["ident"]
    outp = tensors["outp"]

    with ExitStack() as ctx:
        const = ctx.enter_context(tc.tile_pool(name="const", bufs=1))
        wres = ctx.enter_context(tc.tile_pool(name="wres", bufs=1))
        xt_p = ctx.enter_context(tc.tile_pool(name="xt", bufs=10))
        tb_p = ctx.enter_context(tc.tile_pool(name="tb", bufs=2))
        kt_p = ctx.enter_context(tc.tile_pool(name="kt", bufs=1))
        vt_p = ctx.enter_context(tc.tile_pool(name="vt", bufs=1))
        qt_p = ctx.enter_context(tc.tile_pool(name="qt", bufs=2))
        qs_p = ctx.enter_context(tc.tile_pool(name="qs", bufs=4))
        ex_p = ctx.enter_context(tc.tile_pool(name="ex", bufs=DEPTH + 2))
        cx_p = ctx.enter_context(tc.tile_pool(name="cx", bufs=1))
        rc_p = ctx.enter_context(tc.tile_pool(name="rc", bufs=2))
        rb_p = ctx.enter_context(tc.tile_pool(name="rb", bufs=2))
        os_p = ctx.enter_context(tc.tile_pool(name="os", bufs=4))
        qc_p = ctx.enter_context(tc.tile_pool(name="qc", bufs=4))
        pp = ctx.enter_context(tc.tile_pool(name="pp", bufs=2, space="PSUM"))
        ps = ctx.enter_context(tc.tile_pool(name="ps", bufs=2, space="PSUM"))
        pc = ctx.enter_context(tc.tile_pool(name="pc", bufs=1, space="PSUM"))

        # ---- initial DMA: first x super-block + K weights first (first
        # projection is K), spread over four queues; everything else behind.
        xts0 = []
        for ct in range(8):
            xt_t = xt_p.tile([128, 512], bf16, tag="xt", name="xt0")
            eng = nc.gpsimd if ct % 2 == 0 else nc.sync
            eng.dma_start(xt_t[:], xT[ct * 128 : (ct + 1) * 128, 0:512])
            xts0.append(xt_t)
        wq_t = wres.tile([128, 8, DL], bf16, tag="wq")
        wk_t = wres.tile([128, 8, DL], bf16, tag="wk")
        wv_t = wres.tile([128, 8, DL], bf16, tag="wv")
        wo_t = wres.tile([128, 4, D], bf16, tag="wo")
        # K weights all on the (otherwise idle at startup) ACT queue so the
        # first projection can start as soon as chunk 0 lands; Pool stays
        # clear for the rope staging copies.
        for ct in range(8):
            nc.scalar.dma_start(wk_t[:, ct, :], wkT[ct * 128 : (ct + 1) * 128, :])
        tb_t0 = tb_p.tile([128, 2, 512], f32, tag="tb", name="tb0")
        nc.sync.dma_start(tb_t0[:], tabs[:, :, 0:512])
        for ct in range(8):
            eng = nc.sync if ct % 2 == 0 else nc.scalar
            eng.dma_start(wq_t[:, ct, :], wqT[ct * 128 : (ct + 1) * 128, :])
        for ct in range(8):
            eng = nc.sync if ct % 2 == 0 else nc.scalar
            eng.dma_start(wv_t[:, ct, :], wvT[ct * 128 : (ct + 1) * 128, :])
        maskT_t = const.tile([128, 128], bf16, tag="maskT")
        nc.sync.dma_start(maskT_t[:], maskT[:])
        ident_t = const.tile([128, 128], bf16, tag="ident")
        nc.sync.dma_start(ident_t[:], ident[:])
        # wo is not needed until the first pumped out-projection (~mid attn 1)
        woT_r = woT.rearrange("(t p) o -> p t o", p=128)
        for it in range(4):
            nc.scalar.dma_start(wo_t[:, it, :], woT_r[:, it, :])
        # selector for the one-matmul reciprocal broadcast: contraction over
        # partitions 64 (par0 sum) and 0 (par1 sum), accumulated scatters par0's
        # reciprocal onto out rows 0-63 and par1's onto rows 64-127.
        sel_t = const.tile([65, 128], bf16, tag="sel")
        nc.vector.memset(sel_t[0:1, :], 0.0)
        nc.vector.memset(sel_t[64:65, :], 0.0)
        nc.vector.memset(sel_t[0:1, 64:128], 1.0)
        nc.vector.memset(sel_t[64:65, 0:64], 1.0)

        # ---- persistent K / V buffers (one rep's worth, reused across reps)
        kt_tiles = {}
        for hp in range(4):
            for sbk in range(NSB):
                kt_tiles[hp, sbk] = kt_p.tile(
                    [128, 512], bf16, tag=f"kt{hp}_{sbk}", name=f"kt{hp}_{sbk}"
                )
        # V stationaries: even heads (par0) as [dims x64, ones] -> ctx rows
        # 0-63, sum row 64; odd heads (par1) as [ones, zeros x63, dims x64]
        # -> sum row 0, ctx rows 64-127 (lands directly in the upper cxt
        # partitions, no shift; zero rows 1-63 are never read).
        ve_tiles, vo_tiles = {}, {}
        for kb in range(NKB):
            ve_tiles[kb] = vt_p.tile([128, 4, 65], bf16, tag=f"ve{kb}", name=f"ve{kb}")
            nc.vector.memset(ve_tiles[kb][:, :, 64:65], 1.0)
            vo_tiles[kb] = vt_p.tile([128, 4, 128], bf16, tag=f"vo{kb}", name=f"vo{kb}")
            nc.vector.memset(vo_tiles[kb][:, :, 0:64], 0.0)
            nc.vector.memset(vo_tiles[kb][:, :, 0:1], 1.0)

        # ---- deferred-work queue: labeled closures emitted into attention
        work_q = []  # list[(label, closure)]
        emitted = set()

        def pump(n=1):
            for _ in range(n):
                if not work_q:
                    return
                label, fn = work_q.pop(0)
                fn()
                emitted.add(label)

        def pump_until(label):
            while label not in emitted and work_q:
                pump()

        def qk_group(step, ft, is_k, xts, tb_t, psum=None):
            # one 128-feature chunk of the Q or K projection + RoPE. The PSUM
            # result is staged to SBUF bf16 by Pool so every DVE RoPE op runs
            # in the packed-16-bit fast mode.
            w_t = wk_t if is_k else wq_t
            sb = step % NSB

            def emit():
                pool, tag = psum if psum else (pp, "mm")
                pr = pool.tile([128, 512], f32, tag=tag)
                for ct in range(8):
                    nc.tensor.matmul(
                        pr[:],
                        w_t[:, ct, ft * 128 : (ft + 1) * 128],
                        xts[ct][:],
                        start=(ct == 0),
                        stop=(ct == 7),
                    )
                # rope split across DVE (the two PSUM readers) and Pool (the
                # two SBUF-only ops); GPSIMD cannot touch PSUM.
                qs_t = qs_p.tile([128, 512], f32, tag="qs")
                nc.vector.stream_shuffle(qs_t[:], pr[:], SWAP)
                nc.gpsimd.tensor_mul(qs_t[:], qs_t[:], tb_t[:, 1, :])
                qc_t = qc_p.tile([128, 512], f32, tag="qc")
                nc.vector.tensor_mul(qc_t[:], pr[:], tb_t[:, 0, :])
                if is_k:
                    dst = kt_tiles[ft, sb]
                else:
                    dst = qt_p.tile([128, 512], bf16, tag=f"qt{ft}", name=f"qt{ft}")
                    qt_tiles_by_step[step][ft] = dst
                nc.gpsimd.tensor_add(dst[:], qc_t[:], qs_t[:])

            return emit

        def v_group(step, i, xts):
            kb = (step % NSB) * 4 + i

            def emit():
                pr = pp.tile([128, 512], f32, tag="mm")
                for ct in range(8):
                    nc.tensor.matmul(
                        pr[:],
                        xts[ct][:, i * 128 : (i + 1) * 128],
                        wv_t[:, ct, :],
                        start=(ct == 0),
                        stop=(ct == 7),
                    )
                pr_r = pr[:].rearrange("p (a q) -> p a q", q=128)
                nc.vector.tensor_copy(ve_tiles[kb][:, :, 0:64], pr_r[:, :, 0:64])
                nc.vector.tensor_copy(vo_tiles[kb][:, :, 64:128], pr_r[:, :, 64:128])

            return emit

        def out_group(s0, cx_tiles, ob, sq):
            def emit():
                opp = pp.tile([128, 512], f32, tag="mm", name="opp")
                for hp in range(4):
                    nc.tensor.matmul(
                        opp[:],
                        cx_tiles[hp][:, sq * 128 : (sq + 1) * 128],
                        wo_t[:, hp, ob * 512 : (ob + 1) * 512],
                        start=(hp == 0),
                        stop=(hp == 3),
                    )
                ost = os_p.tile([128, 512], f32, tag="os", name="ost")
                nc.scalar.copy(ost[:], opp[:])
                eng = nc.sync if (ob * 4 + sq) % 2 == 0 else nc.gpsimd
                eng.dma_start(
                    outp[s0 + sq * 128 : s0 + (sq + 1) * 128, ob * 512 : (ob + 1) * 512],
                    ost[:],
                )

            return emit

        def queue_projections(step, xts, tb_t):
            # K and V groups for `step` can be emitted any time; Q groups are
            # queued per-hp while the previous step's attention runs (the qt
            # tag is only released once that hp's scores are done).
            for ft in range(4):
                work_q.append(((step, "k", ft), qk_group(step, ft, True, xts, tb_t)))
            for i in range(4):
                work_q.append(((step, "v", i), v_group(step, i, xts)))

        def issue_x_dma(step):
            s0 = (step % NSB) * 512
            xts = []
            for ct in range(8):
                xt_t = xt_p.tile([128, 512], bf16, tag="xt")
                eng = nc.gpsimd if ct % 2 == 0 else nc.sync
                eng.dma_start(xt_t[:], xT[ct * 128 : (ct + 1) * 128, s0 : s0 + 512])
                xts.append(xt_t)
            tb_t = tb_p.tile([128, 2, 512], f32, tag="tb", name="tb")
            nc.sync.dma_start(tb_t[:], tabs[:, :, s0 : s0 + 512])
            return xts, tb_t

        qt_tiles_by_step = {}

        # ---- step 0 projections emitted directly (nothing to hide behind);
        # the attention PSUM pools are still idle here, so alternate the
        # projection accumulators into the score pool to double the number of
        # groups in flight (the 2-slot pp rotation would stall PE otherwise).
        qt_tiles_by_step[0] = {}
        for ft in range(4):
            qk_group(0, ft, True, xts0, tb_t0,
                     psum=None if ft % 2 == 0 else (ps, "sc"))()
            emitted.add((0, "k", ft))
        for ft in range(4):
            qk_group(0, ft, False, xts0, tb_t0,
                     psum=None if ft % 2 == 0 else (ps, "sc"))()
            emitted.add((0, "q", ft))
        for i in range(4):
            v_group(0, i, xts0)()
            emitted.add((0, "v", i))

        # ---- main software-pipelined loop over (rep, super-block) steps
        for step in range(NSTEP):
            sb = step % NSB
            s0 = sb * 512

            if step + 1 < NSTEP:
                xts_n, tb_n = issue_x_dma(step + 1)
                qt_tiles_by_step[step + 1] = {}
                queue_projections(step + 1, xts_n, tb_n)

            qt_tiles = qt_tiles_by_step[step]

            # make sure this step's K/V tiles are in the instruction stream
            for ft in range(4):
                pump_until((step, "k", ft))
            for i in range(4):
                pump_until((step, "v", i))

            n_kj = 4 * (sb + 1)
            cx_tiles = {}
            pending_norm = [None]

            def emit_recips(cxs_):
                # the reciprocals only need the finished cxs sums; emitting
                # them at the end of the owning head-pair shortens the PSUM
                # slot hold inside emit_norm. par0's sum is on partition 64,
                # par1's on partition 0 -> both into one rc tile.
                rc_t = rc_p.tile([65, 512], bf16, tag="rc", name="rc")
                with nc.allow_low_precision(reason="softmax reciprocal"):
                    nc.vector.reciprocal(rc_t[64:65, :], cxs_[0][64:65, :])
                    nc.vector.reciprocal(rc_t[0:1, :], cxs_[1][0:1, :])
                return rc_t

            def emit_norm(tail=False):
                if pending_norm[0] is None:
                    return
                hp_, cxs_, rc_t = pending_norm[0]
                pending_norm[0] = None
                cxt = cx_p.tile([128, 512], bf16, tag=f"cx{hp_}", name=f"cx{hp_}")
                cx_tiles[hp_] = cxt
                # single selector-matmul scatters par0's reciprocal onto rows
                # 0-63 and par1's onto rows 64-127 (contraction over the two
                # sum partitions), drained by one DVE copy; the muls are then
                # fully partition-aligned with the cxs layouts.
                rbp_t = ps.tile([128, 512], f32, tag="sc", name="rbp")
                nc.tensor.matmul(
                    rbp_t[:], sel_t[64:65, :], rc_t[64:65, :], start=True, stop=False
                )
                nc.tensor.matmul(
                    rbp_t[:], sel_t[0:1, :], rc_t[0:1, :], start=False, stop=True
                )
                rb_t = rb_p.tile([128, 512], f32, tag="rb", name="rb")
                nc.vector.tensor_copy(rb_t[:], rbp_t[:])
                if tail:
                    # chunk the final muls so each output-projection column
                    # block can start as soon as its slice is normalized
                    for sq in range(4):
                        c = slice(sq * 128, (sq + 1) * 128)
                        nc.vector.tensor_mul(cxt[0:64, c], cxs_[0][0:64, c], rb_t[0:64, c])
                        nc.vector.tensor_mul(cxt[64:128, c], cxs_[1][64:128, c], rb_t[64:128, c])
                else:
                    nc.vector.tensor_mul(cxt[0:64, :], cxs_[0][0:64, :], rb_t[0:64, :])
                    nc.vector.tensor_mul(cxt[64:128, :], cxs_[1][64:128, :], rb_t[64:128, :])

            for hp in range(4):
                pump_until((step, "q", hp))
                cxs = (
                    pc.tile([65, 512], f32, tag="ce", name="ce"),
                    pc.tile([128, 512], f32, tag="co", name="co"),
                )
                exts = {}
                spans = {}

                def emit_v(kj):
                    qo, w = spans[kj]
                    for par, vt in ((0, ve_tiles[kj]), (1, vo_tiles[kj])):
                        nc.tensor.matmul(
                            cxs[par][:, qo : qo + w],
                            vt[:, hp, :],
                            exts[kj][:, par, 0:w],
                            start=(kj == 0),
                            stop=(kj == n_kj - 1),
                        )
                    del exts[kj]

                for kj in range(n_kj):
                    diag = kj >= 4 * sb
                    kjl = kj - 4 * sb
                    w = 512 - 128 * kjl if diag else 512
                    qo = 512 - w
                    sbk, col = kj // 4, (kj % 4) * 128
                    spans[kj] = (qo, w)
                    scp = ps.tile([128, 2, 512], f32, tag="sc", name="sc")
                    for par in (0, 1):
                        bp = 64 * par
                        kt_sl = kt_tiles[hp, sbk][bp : bp + 64, col : col + 128]
                        qt_sl = qt_tiles[hp][bp : bp + 64, qo : qo + w]
                        if diag:
                            nc.tensor.matmul(
                                scp[:, par, qo : qo + w], kt_sl, qt_sl, start=True, stop=False
                            )
                            nc.tensor.matmul(
                                scp[:, par, qo : qo + 128],
                                ident_t[:],
                                maskT_t[:],
                                start=False,
                                stop=True,
                            )
                        else:
                            nc.tensor.matmul(
                                scp[:, par, :], kt_sl, qt_sl, start=True, stop=True
                            )
                    ext = ex_p.tile([128, 2, 512], bf16, tag="ex", name="ex")
                    nc.scalar.activation(
                        ext[:, :, 0:w], scp[:, :, qo : qo + w], AF.Exp, scale=0.125
                    )
                    exts[kj] = ext
                    if kj == 1:
                        emit_norm()
                    # pump aggressively right after the hp transition (covers
                    # the previous norm's latency), then taper so deferred
                    # work survives into the late, queue-starved phases.
                    lo = 4 if hp == 0 else 2
                    if kj >= lo and (kj <= lo + 2 or kj % 2 == 0):
                        pump(1)
                    if kj >= DEPTH:
                        emit_v(kj - DEPTH)
                for kj in range(max(0, n_kj - DEPTH), n_kj):
                    emit_v(kj)
                emit_norm()  # no-op except flushing when n_kj < 2
                pending_norm[0] = (hp, cxs, emit_recips(cxs))
                if step + 1 < NSTEP:
                    # queue the Q projection chunk whose qt tag just freed
                    work_q.append(
                        ((step + 1, "q", hp), qk_group(step + 1, hp, False, xts_n, tb_n))
                    )
            emit_norm(tail=(step == NSTEP - 1))

            # queue this step's output projection; it interleaves into the
            # next step's attention (PSUM-accumulated, DMA'd straight out).
            for ob in range(2):
                for sq in range(4):
                    work_q.append(
                        ((step, "o", ob * 4 + sq), out_group(s0, cx_tiles, ob, sq))
                    )
            if step + 1 >= NSTEP:
                while work_q:
                    pump()


def build(S=S_FULL, reps=1, chain=False):
    import concourse.tile as tile
    from concourse import bacc, mybir

    f32 = mybir.dt.float32
    bf16 = mybir.dt.bfloat16
    nc = bacc.Bacc(None, target_bir_lowering=False, debug=False)
    t = {}
    t["xT"] = nc.dram_tensor("xT", [D, S], bf16, kind="ExternalInput")
    t["wqT"] = nc.dram_tensor("wqT", [D, DL], bf16, kind="ExternalInput")
    t["wkT"] = nc.dram_tensor("wkT", [D, DL], bf16, kind="ExternalInput")
    t["wvT"] = nc.dram_tensor("wvT", [D, DL], bf16, kind="ExternalInput")
    t["woT"] = nc.dram_tensor("woT", [DL, D], bf16, kind="ExternalInput")
    t["tabs"] = nc.dram_tensor("tabs", [128, 2, S], f32, kind="ExternalInput")
    t["maskT"] = nc.dram_tensor("maskT", [128, 128], bf16, kind="ExternalInput")
    t["ident"] = nc.dram_tensor("ident", [128, 128], bf16, kind="ExternalInput")
    t["outp"] = nc.dram_tensor("outp", [S, D], f32, kind="ExternalOutput")
    if chain:
        t["chain"] = nc.dram_tensor("chain", [128, 128], f32, kind="ExternalInput")
        t["chain_out"] = nc.dram_tensor("chain_out", [128, 128], f32, kind="ExternalOutput")

    with tile.TileContext(nc) as tc:
        _emit(nc, tc, t, S, reps=reps)
        if chain:
            with tc.tile_pool(name="chp", bufs=1) as chp:
                cht = chp.tile([128, 128], mybir.dt.float32, name="cht")
                nc.sync.dma_start(cht[:], t["chain"][:])
                nc.sync.dma_start(t["chain_out"][:], cht[:])
    nc.compile()
    return nc


def prep_inputs(x, Wq, Wk, Wv, Wo, token_positions, S=S_FULL):
    import ml_dtypes

    bf = ml_dtypes.bfloat16
    x = np.asarray(x)
    Wq, Wk, Wv, Wo = (np.asarray(a) for a in (Wq, Wk, Wv, Wo))
    pos = np.asarray(token_positions).astype(np.float64)
    inv = ROPE_THETA ** (-np.arange(0, DK, 2, dtype=np.float64) / DK)  # [32]
    ang = pos[:, None] * inv[None, :]  # [S, 32]
    cos = np.cos(ang).astype(np.float32).T  # [32, S]
    sin = np.sin(ang).astype(np.float32).T
    i_of_p = (np.arange(128) % 64) // 2
    c2 = cos[i_of_p, :]  # [128, S]
    sgn = np.where(np.arange(128) % 2 == 0, -1.0, 1.0).astype(np.float32)
    s2m = sin[i_of_p, :] * sgn[:, None]
    tabs = np.ascontiguousarray(np.stack([c2, s2m], axis=1))  # [128, 2, S]

    maskT = np.where(
        np.arange(128)[None, :] >= np.arange(128)[:, None], 0.0, NEG
    ).astype(bf)
    ident = np.eye(128, dtype=bf)

    nb = x.shape[0]
    maps = []
    for c in range(2 * nb):
        b, half = c // 2, c % 2
        rows = slice(half * DL, (half + 1) * DL)
        maps.append(
            {
                "xT": np.ascontiguousarray(x[b].T).astype(bf),
                "wqT": np.ascontiguousarray(Wq[rows].T).astype(bf),
                "wkT": np.ascontiguousarray(Wk[rows].T).astype(bf),
                "wvT": np.ascontiguousarray(Wv[rows].T).astype(bf),
                "woT": np.ascontiguousarray(Wo[:, rows].T).astype(bf),
                "tabs": tabs,
                "maskT": maskT,
                "ident": ident,
            }
        )
    return maps


def kernel(x, Wq, Wk, Wv, Wo, token_positions):
    from concourse.bass_utils import run_bass_kernel_spmd

    if "nc" not in _CACHE:
        _CACHE["nc"] = build()
    maps = prep_inputs(x, Wq, Wk, Wv, Wo, token_positions)
    res = run_bass_kernel_spmd(_CACHE["nc"], maps, list(range(8)))
    out = np.empty((B, S_FULL, D), np.float32)
    for b in range(B):
        out[b] = res.results[2 * b]["outp"] + res.results[2 * b + 1]["outp"]
    return out
